# revision 10
# baseline (speedup 1.0000x reference)
"""GRU decoder kernel for Trainium2 (8 NeuronCores, data-parallel over batch).

Math (PyTorch GRU, gate order r,z,n), per batch element:
    gx_t = x_t * w_ih + b_ih              (input dim == 1 -> rank-1)
    gh_t = h_{t-1} @ w_hh.T + b_hh
    r = sigmoid(gx_r + gh_r); z = sigmoid(gx_z + gh_z)
    n = tanh(gx_n + b_ih_n + r * (gh_n + b_hh_n))
    h_t = (1-z)*n + z*h_{t-1}
    out = h_T @ fc_w.T + fc_b

Device layout (per core, B_c = 1024 batch):
  - H [128, 512] f16: partitions 0-63 = hidden coords for batch 0-511 (u),
    partitions 64-127 = hidden for batch 512-1023 (v); free dim = batch.
  - Two phase-shifted batch groups (free-dim halves of 256) pipeline the
    per-step chain; 2 PSUM banks per group, double-buffered = all 8 banks.
  - Per group-step, PSUM bank A holds [R | Z] pre-activations (free 0:256 =
    r, 256:512 = z), bank B holds [NH | NX].  One sigmoid ACT op covers both
    r and z; biases b_r/b_z ride into PSUM through a ones-row in the X tile
    (blocks of 63 timesteps + 1 ones row), multiplied by a bias row in the
    one-hot lhsT.
  - Matmuls use all four 64x64 PE quadrants: h-matmuls on row-quadrant of H,
    x-matmuls on the opposite row-quadrant (X stored partition-swapped:
    v-half on partitions 0-63, u-half on 64-127).
  - DVE chain per group-step: STT (hn+bnh)*r, add xn, then three f16 2x-mode
    tensor ops for h' = n + z*(h-n).
"""

import os
import sys

sys.path.insert(0, "/opt/trn_rl_repo")

import numpy as np
from contextlib import ExitStack

HIDDEN = 64
OUT = 256
B = 8192
T = int(os.environ.get("GRU_T", 1024))
NCORES = 8
BC = B // NCORES          # 1024 batch per core
HB = BC // 2              # 512 batch per partition-half
UNROLL = 63               # timesteps per block (row 63 of each half = ones)
NFULL = T // UNROLL       # full blocks
TAIL = T - NFULL * UNROLL # leftover steps in final block
NBLK = NFULL + (1 if TAIL else 0)
NGROUP = 2                # phase-shifted batch groups per core
HG = HB // NGROUP         # 256 free-dim columns per group

_CACHE = {}


def _build():
    import concourse.bass as bass
    import concourse.tile as tile
    from concourse import bacc, mybir

    f16 = mybir.dt.float16
    f32 = mybir.dt.float32
    AF = mybir.ActivationFunctionType
    OP = mybir.AluOpType

    nc = bacc.Bacc("TRN2", target_bir_lowering=False, debug=False,
                   num_devices=NCORES)

    d_x = nc.dram_tensor("xt", [128, NBLK, HB], f16, kind="ExternalInput").ap()
    d_wr = nc.dram_tensor("wr", [128, 64], f16, kind="ExternalInput").ap()
    d_wz = nc.dram_tensor("wz", [128, 64], f16, kind="ExternalInput").ap()
    d_wn = nc.dram_tensor("wn", [128, 64], f16, kind="ExternalInput").ap()
    d_ohr = nc.dram_tensor("ohr", [128, UNROLL, 64], f16, kind="ExternalInput").ap()
    d_ohz = nc.dram_tensor("ohz", [128, UNROLL, 64], f16, kind="ExternalInput").ap()
    d_ohn = nc.dram_tensor("ohn", [128, UNROLL, 64], f16, kind="ExternalInput").ap()
    d_bnh = nc.dram_tensor("bnh", [128, 1], f32, kind="ExternalInput").ap()
    d_bni = nc.dram_tensor("bni", [128, 1], f32, kind="ExternalInput").ap()
    d_fcw = nc.dram_tensor("fcw", [128, OUT], f16, kind="ExternalInput").ap()
    d_fcb = nc.dram_tensor("fcb", [128, 2], f32, kind="ExternalInput").ap()
    d_out = nc.dram_tensor("out", [OUT, BC], f32, kind="ExternalOutput").ap()

    with tile.TileContext(nc) as tc, ExitStack() as ctx:
        singles = ctx.enter_context(tc.tile_pool(name="singles", bufs=1))
        work = ctx.enter_context(tc.tile_pool(name="work", bufs=4))
        psum = ctx.enter_context(tc.tile_pool(name="psum", bufs=1, space="PSUM"))

        X = singles.tile([128, NBLK, HB], f16)
        WR = singles.tile([128, 64], f16)
        WZ = singles.tile([128, 64], f16)
        WN = singles.tile([128, 64], f16)
        OHR = singles.tile([128, UNROLL, 64], f16)
        OHZ = singles.tile([128, UNROLL, 64], f16)
        OHN = singles.tile([128, UNROLL, 64], f16)
        BNH = singles.tile([128, 1], f32)
        BNI = singles.tile([128, 1], f32)
        FCW = singles.tile([128, OUT], f16)
        FCB = singles.tile([128, 2], f32)
        H = singles.tile([128, HB], f16)

        for dst, src in ((X, d_x), (WR, d_wr), (WZ, d_wz), (WN, d_wn),
                         (OHR, d_ohr), (OHZ, d_ohz), (OHN, d_ohn),
                         (BNH, d_bnh), (BNI, d_bni),
                         (FCW, d_fcw), (FCB, d_fcb)):
            nc.gpsimd.dma_start(dst[:], src[:])
        nc.vector.memset(H[:], 0.0)

        U = slice(0, 64)      # partitions: u-half of H / out, v-half of X
        V = slice(64, 128)    # partitions: v-half of H / out, u-half of X
        RS = slice(0, HG)          # used free range within each PSUM bank

        def mms(q, xsb, g, PT):
            gsl = slice(g * HG, (g + 1) * HG)
            hu, hv = H[U, gsl], H[V, gsl]
            xu, xv = xsb[V, :, gsl], xsb[U, :, gsl]
            mm = nc.tensor.matmul
            # PT is a 4-bank PSUM tile: bank 0 = R, 1 = Z, 2 = NH, 3 = NX,
            # each using free columns 0:HG.  Each (partition-half, bank) is an
            # independent accumulation region: the h-matmul starts it, the
            # x-matmul (same out AP -> WAW-ordered) accumulates and stops.
            mm(PT[U, 0, RS], WR[U, :], hu, start=True, stop=False, tile_position=(0, 0))
            mm(PT[U, 0, RS], OHR[V, q, :], xu, start=False, stop=True, tile_position=(64, 0))
            mm(PT[V, 0, RS], WR[V, :], hv, start=True, stop=False, tile_position=(64, 64))
            mm(PT[V, 0, RS], OHR[U, q, :], xv, start=False, stop=True, tile_position=(0, 64))
            mm(PT[U, 1, RS], WZ[U, :], hu, start=True, stop=False, tile_position=(0, 0))
            mm(PT[U, 1, RS], OHZ[V, q, :], xu, start=False, stop=True, tile_position=(64, 0))
            mm(PT[V, 1, RS], WZ[V, :], hv, start=True, stop=False, tile_position=(64, 64))
            mm(PT[V, 1, RS], OHZ[U, q, :], xv, start=False, stop=True, tile_position=(0, 64))
            mm(PT[U, 2, RS], WN[U, :], hu, start=True, stop=True, tile_position=(0, 0))
            mm(PT[V, 2, RS], WN[V, :], hv, start=True, stop=True, tile_position=(64, 64))
            mm(PT[U, 3, RS], OHN[V, q, :], xu, start=True, stop=True, tile_position=(64, 0))
            mm(PT[V, 3, RS], OHN[U, q, :], xv, start=True, stop=True, tile_position=(0, 64))

        def elem(g, PT):
            gsl = slice(g * HG, (g + 1) * HG)
            SG = work.tile([128, 2, HG], f16, tag=f"SG{g}")
            T1 = work.tile([128, HG], f16, tag=f"T1{g}")
            T2 = work.tile([128, HG], f16, tag=f"T2{g}")
            UU = work.tile([128, HG], f16, tag=f"UU{g}")
            VV = work.tile([128, HG], f16, tag=f"VV{g}")
            # r|z = sigmoid(banks 0-1)  (biases folded in via the ones-row)
            nc.scalar.activation(SG[:], PT[:, 0:2, RS], AF.Sigmoid)
            # T1 = (hn + b_hh_n) * r
            nc.vector.scalar_tensor_tensor(T1[:], PT[:, 2, RS], BNH[:],
                                           SG[:, 0, :], op0=OP.add, op1=OP.mult)
            # T2 = T1 + xn
            nc.vector.tensor_add(T2[:], T1[:], PT[:, 3, RS])
            # n = tanh(T2 + b_ih_n)
            NN = work.tile([128, HG], f16, tag=f"NN{g}")
            nc.scalar.activation(NN[:], T2[:], AF.Tanh, bias=BNI[:])
            # h' = n + z*(h - n)
            nc.vector.tensor_sub(UU[:], H[:, gsl], NN[:])
            nc.vector.tensor_mul(VV[:], SG[:, 1, :], UU[:])
            nc.vector.tensor_add(H[:, gsl], NN[:], VV[:])

        def body(blk, nstep):
            if isinstance(blk, int):
                blk = slice(blk, blk + 1)
            xsb = X[:, blk, :]
            for q in range(nstep):
                tiles = []
                for g in range(NGROUP):
                    PT = psum.tile([128, 4, HB], f32, tag=f"PT{g}")
                    mms(q, xsb, g, PT)
                    tiles.append(PT)
                for g in range(NGROUP):
                    elem(g, tiles[g])

        if NFULL <= 1 or os.environ.get("GRU_NOHWLOOP"):
            for blk in range(NFULL):
                body(blk, UNROLL)
        elif False:
            pass
        else:
            with tc.For_i(0, NFULL, 1,
                          hint_engines=(mybir.EngineType.PE,)) as i:
                body(bass.ds(i, 1), UNROLL)
        if TAIL:
            body(NFULL, TAIL)

        # Final FC: out[o, b] = sum_k fc_w[o, k] h[b, k] + fc_b[o]
        for oh in range(2):
            osl = slice(oh * 128, (oh + 1) * 128)
            fc_u = psum.tile([128, HB], f32, tag="PT0")
            fc_v = psum.tile([128, HB], f32, tag="PT1")
            nc.tensor.matmul(fc_u[:], FCW[0:64, osl], H[0:64, :],
                             start=True, stop=True, tile_position=(0, 0))
            nc.tensor.matmul(fc_v[:], FCW[64:128, osl], H[64:128, :],
                             start=True, stop=True, tile_position=(64, 0))
            Ou = work.tile([128, HB], f32, tag="Ou")
            Ov = work.tile([128, HB], f32, tag="Ov")
            nc.scalar.activation(Ou[:], fc_u[:], AF.Identity,
                                 bias=FCB[:, oh:oh + 1])
            nc.scalar.activation(Ov[:], fc_v[:], AF.Identity,
                                 bias=FCB[:, oh:oh + 1])
            nc.gpsimd.dma_start(d_out[osl, 0:HB], Ou[:])
            nc.gpsimd.dma_start(d_out[osl, HB:BC], Ov[:])

    nc.compile()
    return nc


def _host_inputs(x, w_ih, w_hh, b_ih, b_hh, fc_w, fc_b):
    """Build the per-core in_maps (numpy, laid out exactly as SBUF tiles)."""
    f16 = np.float16
    f32 = np.float32
    x = np.asarray(x, f32)
    w_ih = np.asarray(w_ih, f32)
    w_hh = np.asarray(w_hh, f32)
    b_ih = np.asarray(b_ih, f32)
    b_hh = np.asarray(b_hh, f32)
    fc_w = np.asarray(fc_w, f32)
    fc_b = np.asarray(fc_b, f32)

    def oh(seg, bias_vec):
        w = w_ih[seg, 0]                            # [64]
        o = np.zeros((64, UNROLL, 64), f32)
        for q in range(UNROLL):
            o[q, q, :] = w
        o[63, :, :] = bias_vec[None, :]             # ones-row coefficient
        return np.concatenate([o, o], 0).astype(f16)  # [128, UNROLL, 64]

    def wstack(seg):
        t = w_hh[seg, :].T                            # [64(k), 64(m)]
        return np.vstack([t, t]).astype(f16)

    def btile(v):
        return np.tile(v.reshape(-1, 1), (2, 1)).astype(f32)  # [128, 1]

    br = b_ih[0:64] + b_hh[0:64]
    bz = b_ih[64:128] + b_hh[64:128]
    shared = {
        "wr": wstack(slice(0, 64)),
        "wz": wstack(slice(64, 128)),
        "wn": wstack(slice(128, 192)),
        "ohr": oh(slice(0, 64), br),
        "ohz": oh(slice(64, 128), bz),
        "ohn": oh(slice(128, 192), np.zeros(64, f32)),
        "bnh": btile(b_hh[128:192]),
        "bni": btile(b_ih[128:192]),
        "fcw": np.vstack([fc_w.T, fc_w.T]).astype(f16),  # [128, 256]
        "fcb": np.stack([fc_b[0:128], fc_b[128:256]], 1).astype(f32),
    }

    in_maps = []
    for c in range(NCORES):
        xs = x[c * BC:(c + 1) * BC, :T, 0]            # [BC b, T t]
        xT = xs.T                                     # [T, BC]
        xpad = np.zeros((NBLK * UNROLL, BC), f32)
        xpad[:T] = xT
        xr = xpad.reshape(NBLK, UNROLL, BC)           # [blk, q, b]
        Xh = np.zeros((128, NBLK, HB), f32)
        Xh[0:UNROLL] = xr[:, :, HB:BC].transpose(1, 0, 2)   # v-half timesteps
        Xh[63] = 1.0                                        # v ones row
        Xh[64:64 + UNROLL] = xr[:, :, 0:HB].transpose(1, 0, 2)  # u-half
        Xh[127] = 1.0                                       # u ones row
        m = dict(shared)
        m["xt"] = np.ascontiguousarray(Xh).astype(f16)
        in_maps.append(m)
    return in_maps


def _run(in_maps, trace=False):
    from concourse import bass_utils
    if "nc" not in _CACHE:
        _CACHE["nc"] = _build()
    nc = _CACHE["nc"]
    res = bass_utils.run_bass_kernel_spmd(
        nc, in_maps, core_ids=list(range(NCORES)), trace=trace)
    return res


def kernel(**inputs):
    in_maps = _host_inputs(**inputs)
    res = _run(in_maps, trace=False)
    out = np.empty([B, OUT], np.float32)
    for c in range(NCORES):
        out[c * BC:(c + 1) * BC, :] = res.results[c]["out"].T
    return out


# revision 12
# speedup vs baseline: 4.7515x; 4.7515x over previous
"""GRU decoder kernel for Trainium2 (8 NeuronCores, data-parallel over batch).

Math (PyTorch GRU, gate order r,z,n), per batch element:
    gx_t = x_t * w_ih + b_ih              (input dim == 1 -> rank-1)
    gh_t = h_{t-1} @ w_hh.T + b_hh
    r = sigmoid(gx_r + gh_r); z = sigmoid(gx_z + gh_z)
    n = tanh(gx_n + b_ih_n + r * (gh_n + b_hh_n))
    h_t = (1-z)*n + z*h_{t-1}
    out = h_T @ fc_w.T + fc_b

Device layout (per core, B_c = 1024 batch):
  - H [128, 512] f16: partitions 0-63 = hidden coords for batch 0-511 (u),
    partitions 64-127 = hidden for batch 512-1023 (v); free dim = batch.
  - Two phase-shifted batch groups (free-dim halves of 256) pipeline the
    per-step chain; 2 PSUM banks per group, double-buffered = all 8 banks.
  - Per group-step, PSUM bank A holds [R | Z] pre-activations (free 0:256 =
    r, 256:512 = z), bank B holds [NH | NX].  One sigmoid ACT op covers both
    r and z; biases b_r/b_z ride into PSUM through a ones-row in the X tile
    (blocks of 63 timesteps + 1 ones row), multiplied by a bias row in the
    one-hot lhsT.
  - Matmuls use all four 64x64 PE quadrants: h-matmuls on row-quadrant of H,
    x-matmuls on the opposite row-quadrant (X stored partition-swapped:
    v-half on partitions 0-63, u-half on 64-127).
  - DVE chain per group-step: STT (hn+bnh)*r, add xn, then three f16 2x-mode
    tensor ops for h' = n + z*(h-n).
"""

import os
import sys

sys.path.insert(0, "/opt/trn_rl_repo")

import numpy as np
from contextlib import ExitStack

HIDDEN = 64
OUT = 256
B = 8192
T = int(os.environ.get("GRU_T", 1024))
NCORES = 8
BC = B // NCORES          # 1024 batch per core
HB = BC // 2              # 512 batch per partition-half
UNROLL = 63               # timesteps per block (row 63 of each half = ones)
NFULL = T // UNROLL       # full blocks
TAIL = T - NFULL * UNROLL # leftover steps in final block
NBLK = NFULL + (1 if TAIL else 0)
NGROUP = 2                # phase-shifted batch groups per core
HG = HB // NGROUP         # 256 free-dim columns per group

_CACHE = {}


def _build():
    import concourse.bass as bass
    import concourse.tile as tile
    from concourse import bacc, mybir

    f16 = mybir.dt.float16
    f32 = mybir.dt.float32
    AF = mybir.ActivationFunctionType
    OP = mybir.AluOpType

    nc = bacc.Bacc("TRN2", target_bir_lowering=False, debug=False,
                   num_devices=NCORES)

    d_x = nc.dram_tensor("xt", [128, NBLK, HB], f16, kind="ExternalInput").ap()
    d_wr = nc.dram_tensor("wr", [128, 64], f16, kind="ExternalInput").ap()
    d_wz = nc.dram_tensor("wz", [128, 64], f16, kind="ExternalInput").ap()
    d_wn = nc.dram_tensor("wn", [128, 64], f16, kind="ExternalInput").ap()
    d_ohr = nc.dram_tensor("ohr", [128, UNROLL, 64], f16, kind="ExternalInput").ap()
    d_ohz = nc.dram_tensor("ohz", [128, UNROLL, 64], f16, kind="ExternalInput").ap()
    d_ohn = nc.dram_tensor("ohn", [128, UNROLL, 64], f16, kind="ExternalInput").ap()
    d_bnh = nc.dram_tensor("bnh", [128, 1], f32, kind="ExternalInput").ap()
    d_bni = nc.dram_tensor("bni", [128, 1], f32, kind="ExternalInput").ap()
    d_fcw = nc.dram_tensor("fcw", [128, OUT], f16, kind="ExternalInput").ap()
    d_fcb = nc.dram_tensor("fcb", [128, 2], f32, kind="ExternalInput").ap()
    d_out = nc.dram_tensor("out", [OUT, BC], f32, kind="ExternalOutput").ap()

    with tile.TileContext(nc) as tc, ExitStack() as ctx:
        singles = ctx.enter_context(tc.tile_pool(name="singles", bufs=1))
        work = ctx.enter_context(tc.tile_pool(name="work", bufs=4))
        psum = ctx.enter_context(tc.tile_pool(name="psum", bufs=1, space="PSUM"))

        X = singles.tile([128, NBLK, HB], f16)
        WR = singles.tile([128, 64], f16)
        WZ = singles.tile([128, 64], f16)
        WN = singles.tile([128, 64], f16)
        OHR = singles.tile([128, UNROLL, 64], f16)
        OHZ = singles.tile([128, UNROLL, 64], f16)
        OHN = singles.tile([128, UNROLL, 64], f16)
        BNH = singles.tile([128, 1], f32)
        BNI = singles.tile([128, 1], f32)
        FCW = singles.tile([128, OUT], f16)
        FCB = singles.tile([128, 2], f32)
        H = singles.tile([128, HB], f16)

        for dst, src in ((X, d_x), (WR, d_wr), (WZ, d_wz), (WN, d_wn),
                         (OHR, d_ohr), (OHZ, d_ohz), (OHN, d_ohn),
                         (BNH, d_bnh), (BNI, d_bni),
                         (FCW, d_fcw), (FCB, d_fcb)):
            nc.gpsimd.dma_start(dst[:], src[:])
        nc.vector.memset(H[:], 0.0)

        U = slice(0, 64)      # partitions: u-half of H / out, v-half of X
        V = slice(64, 128)    # partitions: v-half of H / out, u-half of X
        RS = slice(0, HG)          # used free range within each PSUM bank

        no4q = bool(os.environ.get("GRU_NO4Q"))

        def mms(q, xsb, g, PT):
            gsl = slice(g * HG, (g + 1) * HG)
            hu, hv = H[U, gsl], H[V, gsl]
            if no4q:
                # x rows co-located with h halves; x-matmuls share quadrants
                xu, xv = xsb[U, :, gsl], xsb[V, :, gsl]
                ou, ov = U, V
                txu, txv = (0, 0), (64, 64)  # share h quadrants
            else:
                xu, xv = xsb[V, :, gsl], xsb[U, :, gsl]
                ou, ov = V, U
                txu, txv = (64, 0), (0, 64)
            mm = nc.tensor.matmul
            # PT is a 4-bank PSUM tile: bank 0 = R, 1 = Z, 2 = NH, 3 = NX,
            # each using free columns 0:HG.  Each (partition-half, bank) is an
            # independent accumulation region: the h-matmul starts it, the
            # x-matmul (same out AP -> WAW-ordered) accumulates and stops.
            mm(PT[U, 0, RS], WR[U, :], hu, start=True, stop=False, tile_position=(0, 0))
            mm(PT[U, 0, RS], OHR[ou, q, :], xu, start=False, stop=True, tile_position=txu)
            mm(PT[V, 0, RS], WR[V, :], hv, start=True, stop=False, tile_position=(64, 64))
            mm(PT[V, 0, RS], OHR[ov, q, :], xv, start=False, stop=True, tile_position=txv)
            mm(PT[U, 1, RS], WZ[U, :], hu, start=True, stop=False, tile_position=(0, 0))
            mm(PT[U, 1, RS], OHZ[ou, q, :], xu, start=False, stop=True, tile_position=txu)
            mm(PT[V, 1, RS], WZ[V, :], hv, start=True, stop=False, tile_position=(64, 64))
            mm(PT[V, 1, RS], OHZ[ov, q, :], xv, start=False, stop=True, tile_position=txv)
            mm(PT[U, 2, RS], WN[U, :], hu, start=True, stop=True, tile_position=(0, 0))
            mm(PT[V, 2, RS], WN[V, :], hv, start=True, stop=True, tile_position=(64, 64))
            mm(PT[U, 3, RS], OHN[ou, q, :], xu, start=True, stop=True, tile_position=txu)
            mm(PT[V, 3, RS], OHN[ov, q, :], xv, start=True, stop=True, tile_position=txv)

        def elem(g, PT):
            gsl = slice(g * HG, (g + 1) * HG)
            SG = work.tile([128, 2, HG], f16, tag=f"SG{g}")
            T1 = work.tile([128, HG], f16, tag=f"T1{g}")
            T2 = work.tile([128, HG], f16, tag=f"T2{g}")
            UU = work.tile([128, HG], f16, tag=f"UU{g}")
            VV = work.tile([128, HG], f16, tag=f"VV{g}")
            # r|z = sigmoid(banks 0-1)  (biases folded in via the ones-row)
            nc.scalar.activation(SG[:], PT[:, 0:2, RS], AF.Sigmoid)
            # T1 = (hn + b_hh_n) * r
            nc.vector.scalar_tensor_tensor(T1[:], PT[:, 2, RS], BNH[:],
                                           SG[:, 0, :], op0=OP.add, op1=OP.mult)
            # T2 = T1 + xn
            nc.vector.tensor_add(T2[:], T1[:], PT[:, 3, RS])
            # n = tanh(T2 + b_ih_n)
            NN = work.tile([128, HG], f16, tag=f"NN{g}")
            nc.scalar.activation(NN[:], T2[:], AF.Tanh, bias=BNI[:])
            # h' = n + z*(h - n)
            nc.vector.tensor_sub(UU[:], H[:, gsl], NN[:])
            nc.vector.tensor_mul(VV[:], SG[:, 1, :], UU[:])
            nc.vector.tensor_add(H[:, gsl], NN[:], VV[:])

        def body(blk, nstep):
            if isinstance(blk, int):
                blk = slice(blk, blk + 1)
            xsb = X[:, blk, :]
            for q in range(nstep):
                tiles = []
                for g in range(NGROUP):
                    PT = psum.tile([128, 4, HB], f32, tag=f"PT{g}")
                    mms(q, xsb, g, PT)
                    tiles.append(PT)
                for g in range(NGROUP):
                    elem(g, tiles[g])

        if NFULL <= 1 or os.environ.get("GRU_NOHWLOOP"):
            for blk in range(NFULL):
                body(blk, UNROLL)
        elif False:
            pass
        else:
            with tc.For_i(0, NFULL, 1,
                          hint_engines=(mybir.EngineType.PE,)) as i:
                body(bass.ds(i, 1), UNROLL)
        if TAIL:
            body(NFULL, TAIL)

        # Final FC: out[o, b] = sum_k fc_w[o, k] h[b, k] + fc_b[o]
        for oh in range(2):
            osl = slice(oh * 128, (oh + 1) * 128)
            fc_u = psum.tile([128, HB], f32, tag="PT0")
            fc_v = psum.tile([128, HB], f32, tag="PT1")
            nc.tensor.matmul(fc_u[:], FCW[0:64, osl], H[0:64, :],
                             start=True, stop=True, tile_position=(0, 0))
            nc.tensor.matmul(fc_v[:], FCW[64:128, osl], H[64:128, :],
                             start=True, stop=True, tile_position=(64, 0))
            Ou = work.tile([128, HB], f32, tag="Ou")
            Ov = work.tile([128, HB], f32, tag="Ov")
            nc.scalar.activation(Ou[:], fc_u[:], AF.Identity,
                                 bias=FCB[:, oh:oh + 1])
            nc.scalar.activation(Ov[:], fc_v[:], AF.Identity,
                                 bias=FCB[:, oh:oh + 1])
            nc.gpsimd.dma_start(d_out[osl, 0:HB], Ou[:])
            nc.gpsimd.dma_start(d_out[osl, HB:BC], Ov[:])

    nc.compile()
    return nc


def _host_inputs(x, w_ih, w_hh, b_ih, b_hh, fc_w, fc_b):
    """Build the per-core in_maps (numpy, laid out exactly as SBUF tiles)."""
    f16 = np.float16
    f32 = np.float32
    x = np.asarray(x, f32)
    w_ih = np.asarray(w_ih, f32)
    w_hh = np.asarray(w_hh, f32)
    b_ih = np.asarray(b_ih, f32)
    b_hh = np.asarray(b_hh, f32)
    fc_w = np.asarray(fc_w, f32)
    fc_b = np.asarray(fc_b, f32)

    def oh(seg, bias_vec):
        w = w_ih[seg, 0]                            # [64]
        o = np.zeros((64, UNROLL, 64), f32)
        for q in range(UNROLL):
            o[q, q, :] = w
        o[63, :, :] = bias_vec[None, :]             # ones-row coefficient
        return np.concatenate([o, o], 0).astype(f16)  # [128, UNROLL, 64]

    def wstack(seg):
        t = w_hh[seg, :].T                            # [64(k), 64(m)]
        return np.vstack([t, t]).astype(f16)

    def btile(v):
        return np.tile(v.reshape(-1, 1), (2, 1)).astype(f32)  # [128, 1]

    br = b_ih[0:64] + b_hh[0:64]
    bz = b_ih[64:128] + b_hh[64:128]
    shared = {
        "wr": wstack(slice(0, 64)),
        "wz": wstack(slice(64, 128)),
        "wn": wstack(slice(128, 192)),
        "ohr": oh(slice(0, 64), br),
        "ohz": oh(slice(64, 128), bz),
        "ohn": oh(slice(128, 192), np.zeros(64, f32)),
        "bnh": btile(b_hh[128:192]),
        "bni": btile(b_ih[128:192]),
        "fcw": np.vstack([fc_w.T, fc_w.T]).astype(f16),  # [128, 256]
        "fcb": np.stack([fc_b[0:128], fc_b[128:256]], 1).astype(f32),
    }

    in_maps = []
    for c in range(NCORES):
        xs = x[c * BC:(c + 1) * BC, :T, 0]            # [BC b, T t]
        xT = xs.T                                     # [T, BC]
        xpad = np.zeros((NBLK * UNROLL, BC), f32)
        xpad[:T] = xT
        xr = xpad.reshape(NBLK, UNROLL, BC)           # [blk, q, b]
        Xh = np.zeros((128, NBLK, HB), f32)
        if os.environ.get("GRU_NO4Q"):
            lo_half, hi_half = xr[:, :, 0:HB], xr[:, :, HB:BC]  # u low, v high
        else:
            lo_half, hi_half = xr[:, :, HB:BC], xr[:, :, 0:HB]  # v low, u high
        Xh[0:UNROLL] = lo_half.transpose(1, 0, 2)
        Xh[63] = 1.0
        Xh[64:64 + UNROLL] = hi_half.transpose(1, 0, 2)
        Xh[127] = 1.0
        m = dict(shared)
        m["xt"] = np.ascontiguousarray(Xh).astype(f16)
        in_maps.append(m)
    return in_maps


def _run(in_maps, trace=False):
    from concourse import bass_utils
    if "nc" not in _CACHE:
        _CACHE["nc"] = _build()
    nc = _CACHE["nc"]
    res = bass_utils.run_bass_kernel_spmd(
        nc, in_maps, core_ids=list(range(NCORES)), trace=trace)
    return res


def kernel(**inputs):
    in_maps = _host_inputs(**inputs)
    res = _run(in_maps, trace=False)
    out = np.empty([B, OUT], np.float32)
    for c in range(NCORES):
        out[c * BC:(c + 1) * BC, :] = res.results[c]["out"].T
    return out
